# revision 13
# baseline (speedup 1.0000x reference)
"""Trainium2 Bass kernel for CrAKNAttention (N=3072, DIN=512, H=16, D=64) on 8 NeuronCores.

Sharding: queries are split 8 ways (384 rows per core); every core computes all 16
heads for its query slice plus the full K/V projections, so no collectives are
needed. All device matmuls run in bf16 with f32 PSUM accumulation.

Layout notes (everything transposed so the PE contraction dim is the partition dim):
 - host passes qT/kT/vT [DIN, nq|N], bias^T/mask^T slices [N, nq], and transposed
   weights; 1/sqrt(D) is folded into Wq on the host.
 - QK^T is computed key-major ([128 keys, 384 queries] tiles); softmax numerator is
   exp(QK) * (exp(bias) * mask) -- exact since logits are bounded far below exp
   overflow and masked entries multiply to 0.
 - attn^T tiles feed the AV matmul directly as lhsT; a ones-column matmul into the
   same PSUM bank accumulates the softmax denominator.
 - o_proj runs on vals^T (Mish is a native ScalarE activation); the core writes the
   transposed output slice [512, 384], which the host transposes back and concats.
"""

import math
import sys

sys.path.insert(0, "/opt/trn_rl_repo")

import numpy as np
import ml_dtypes

import concourse.bass as bass  # noqa: F401  (registers engines)
import concourse.mybir as mybir
import concourse.tile as tile
from concourse import bacc
from concourse.bass_utils import run_bass_kernel_spmd
from concourse.masks import make_identity

f32 = mybir.dt.float32
bf16 = mybir.dt.bfloat16
BF = ml_dtypes.bfloat16

N, DIN, H, Dh = 3072, 512, 16, 64
HD = H * Dh  # 1024
NCORES = 8
NQ = N // NCORES  # 384 queries per core
QTILES = NQ // 128  # 3
KC = N // 128  # 24 key chunks
DC = DIN // 128  # 4 din chunks
MH = HD // 128  # 8 hd tiles
MO = DIN // 128  # 4 output tiles


def _body(tc, A, out, dbg=None, amplify=1):
    nc = tc.nc
    Exp = mybir.ActivationFunctionType.Exp
    Mish = mybir.ActivationFunctionType.Mish
    Ident = mybir.ActivationFunctionType.Identity
    from contextlib import ExitStack

    with ExitStack() as ctx:
        res = ctx.enter_context(tc.tile_pool(name="res", bufs=1))
        dram = ctx.enter_context(tc.tile_pool(name="dram", bufs=1, space="DRAM"))
        ps = ctx.enter_context(tc.tile_pool(name="ps", bufs=8, space="PSUM"))

        # resident SBUF tiles
        wq_s = res.tile([128, DC * HD], bf16, tag="wq")
        wk_s = res.tile([128, DC * HD], bf16, tag="wk")
        wv_s = res.tile([128, DC * HD], bf16, tag="wv")
        wo1_s = res.tile([128, MH * HD], bf16, tag="wo1")
        wo2_s = res.tile([128, MH * DIN], bf16, tag="wo2")
        QT_s = res.tile([128, MH * NQ], bf16, tag="qt")
        V_s = res.tile([128, KC * H * 65], bf16, tag="v")
        eb_s = res.tile([128, KC * NQ], bf16, tag="eb")
        vals_s = res.tile([128, QTILES * HD], bf16, tag="vals")
        valsT_s = res.tile([128, MH * NQ], bf16, tag="valsT")
        bo1_s = res.tile([128, MH], f32, tag="bo1")
        bo2_s = res.tile([128, MO], f32, tag="bo2")
        ident = res.tile([128, 128], bf16, tag="ident")
        ones_s = res.tile([128, 1], bf16, tag="ones")
        ktd = dram.tile([HD, N], bf16)

        make_identity(nc, ident[:])
        nc.vector.memset(ones_s[:], 1.0)
        nc.vector.memset(V_s[:], 1.0)  # ones columns for the softmax denominator

        for c in range(DC):
            nc.sync.dma_start(wq_s[:, c * HD:(c + 1) * HD], A["wq"][c * 128:(c + 1) * 128, :])
            nc.sync.dma_start(wk_s[:, c * HD:(c + 1) * HD], A["wk"][c * 128:(c + 1) * 128, :])
            nc.sync.dma_start(wv_s[:, c * HD:(c + 1) * HD], A["wv"][c * 128:(c + 1) * 128, :])
        for c in range(MH):
            nc.sync.dma_start(wo1_s[:, c * HD:(c + 1) * HD], A["wo1"][c * 128:(c + 1) * 128, :])
            nc.sync.dma_start(wo2_s[:, c * DIN:(c + 1) * DIN], A["wo2"][c * 128:(c + 1) * 128, :])
        nc.sync.dma_start(bo1_s[:], A["bo1"].rearrange("(m p) o -> p (m o)", p=128))
        nc.sync.dma_start(bo2_s[:], A["bo2"].rearrange("(m p) o -> p (m o)", p=128))

        # exp(bias)*mask, key-major [128k, NQ] per key chunk
        ebp = ctx.enter_context(tc.tile_pool(name="ebp", bufs=2))
        for kc in range(KC):
            bt = ebp.tile([128, NQ], bf16, tag="bt")
            mt = ebp.tile([128, NQ], bf16, tag="mt")
            nc.sync.dma_start(bt[:], A["bT"][kc * 128:(kc + 1) * 128, :])
            nc.sync.dma_start(mt[:], A["mT"][kc * 128:(kc + 1) * 128, :])
            et = ebp.tile([128, NQ], bf16, tag="et")
            nc.scalar.activation(et[:], bt[:], Exp)
            nc.vector.tensor_mul(eb_s[:, kc * NQ:(kc + 1) * NQ], et[:], mt[:])

        # q projection -> QT_s [hd, q] (hd-major)
        qp = ctx.enter_context(tc.tile_pool(name="qp", bufs=2))
        qpsums = [ps.tile([128, 512], f32, tag="ps", name=f"qps{m}") for m in range(MH)]
        for c in range(DC):
            qr = qp.tile([128, NQ], bf16, tag="qr")
            nc.sync.dma_start(qr[:], A["qT"][c * 128:(c + 1) * 128, :])
            for m in range(MH):
                nc.tensor.matmul(
                    qpsums[m][:, 0:NQ], wq_s[:, c * HD + m * 128:c * HD + (m + 1) * 128],
                    qr[:], start=(c == 0), stop=(c == DC - 1))
        for m in range(MH):
            nc.scalar.copy(QT_s[:, m * NQ:(m + 1) * NQ], qpsums[m][:, 0:NQ])

        # k projection -> ktd DRAM [hd, tok] (hd-major)
        kp = ctx.enter_context(tc.tile_pool(name="kp", bufs=3))
        kst = ctx.enter_context(tc.tile_pool(name="kst", bufs=3))
        for n in range(N // 512):
            kpsums = [ps.tile([128, 512], f32, tag="ps", name=f"kps{n}_{m}") for m in range(MH)]
            for c in range(DC):
                kr = kp.tile([128, 512], bf16, tag="kr")
                nc.sync.dma_start(kr[:], A["kT"][c * 128:(c + 1) * 128, n * 512:(n + 1) * 512])
                for m in range(MH):
                    nc.tensor.matmul(
                        kpsums[m][:], wk_s[:, c * HD + m * 128:c * HD + (m + 1) * 128], kr[:],
                        start=(c == 0), stop=(c == DC - 1))
            for m in range(MH):
                ks = kst.tile([128, 512], bf16, tag="ks")
                nc.scalar.copy(ks[:], kpsums[m][:])
                nc.sync.dma_start(ktd[m * 128:(m + 1) * 128, n * 512:(n + 1) * 512], ks[:])

        # v projection -> V_s [tok, hd] (token-major)
        vp = ctx.enter_context(tc.tile_pool(name="vp", bufs=3))
        for mtg in range(KC // 4):
            vts = []
            for c in range(DC):
                vt = vp.tile([128, 512], bf16, tag="vt", name=f"vt{mtg}_{c}")
                nc.sync.dma_start(vt[:], A["vT"][c * 128:(c + 1) * 128, mtg * 512:(mtg + 1) * 512])
                vts.append(vt)
            for j in range(4):
                mt = mtg * 4 + j
                for half in range(2):
                    pv = ps.tile([128, 512], f32, tag="ps")
                    for c in range(DC):
                        nc.tensor.matmul(
                            pv[:], vts[c][:, j * 128:(j + 1) * 128],
                            wv_s[:, c * HD + half * 512:c * HD + half * 512 + 512],
                            start=(c == 0), stop=(c == DC - 1))
                    dst = V_s[:].rearrange("p (a h e) -> p a h e", a=KC, h=H)[
                        :, mt, half * 8:(half + 1) * 8, 0:64]
                    nc.vector.tensor_copy(dst, pv[:].rearrange("p (h e) -> p h e", h=8))

        # attention, head by head
        kwp = ctx.enter_context(tc.tile_pool(name="kwp", bufs=2))
        atp = ctx.enter_context(tc.tile_pool(name="atp", bufs=3))
        rcp = ctx.enter_context(tc.tile_pool(name="rcp", bufs=2))
        for rep in range(amplify):
          for h in range(H):
            avs = [ps.tile([128, 65], f32, tag="ps", name=f"av{rep}_{h}_{q}") for q in range(QTILES)]
            qrhs = QT_s[(h % 2) * 64:(h % 2) * 64 + 64, (h // 2) * NQ:(h // 2 + 1) * NQ]
            off = (h % 2) * 64
            kht = kwp.tile([128, N], bf16, tag="kht", name=f"kht{rep}_{h}")
            nc.sync.dma_start(kht[off:off + 64, :], ktd[h * 64:h * 64 + 64, :])
            for kc in range(KC):
                pqk = ps.tile([128, NQ], f32, tag="ps")
                nc.tensor.matmul(
                    pqk[:], kht[off:off + 64, kc * 128:(kc + 1) * 128], qrhs,
                    start=True, stop=True)
                at = atp.tile([128, NQ], bf16, tag="at")
                nc.scalar.activation(at[:], pqk[:], Exp)
                am = atp.tile([128, NQ], bf16, tag="am")
                nc.vector.tensor_mul(am[:], at[:], eb_s[:, kc * NQ:(kc + 1) * NQ])
                if dbg is not None and h == 0 and kc == 0:
                    lgs = atp.tile([128, NQ], f32, tag="lgs")
                    nc.vector.tensor_copy(lgs[:], pqk[:])
                    nc.sync.dma_start(dbg["lg_d"][:], lgs[:])
                    nc.sync.dma_start(dbg["am_d"][:], am[:])
                first, last = kc == 0, kc == KC - 1
                vslice = V_s[:, kc * H * 65 + h * 65:kc * H * 65 + h * 65 + 65]
                for qt in range(QTILES):
                    nc.tensor.matmul(
                        avs[qt][:, 0:65], am[:, qt * 128:(qt + 1) * 128], vslice,
                        start=first, stop=last)
            if dbg is not None and h == 0:
                avd = atp.tile([128, 65], f32, tag="avd")
                nc.vector.tensor_copy(avd[:], avs[0][:])
                nc.sync.dma_start(dbg["av_d"][:], avd[:])
            for qt in range(QTILES):
                rc = rcp.tile([128, 1], f32, tag="rc")
                nc.vector.reciprocal(rc[:], avs[qt][:, 64:65])
                nc.vector.tensor_scalar_mul(
                    vals_s[:, qt * HD + h * 64:qt * HD + h * 64 + 64], avs[qt][:, 0:64], rc[:])

        # vals [q, hd] -> valsT [hd, q] via PE transpose
        for qt in range(QTILES):
            for m in range(MH):
                pt = ps.tile([128, 128], bf16, tag="ps")
                nc.tensor.transpose(pt[:], vals_s[:, qt * HD + m * 128:qt * HD + (m + 1) * 128], ident[:])
                nc.scalar.copy(valsT_s[:, m * NQ + qt * 128:m * NQ + (qt + 1) * 128], pt[:])

        # o_proj: out^T = Wo2 @ mish(Wo1 @ vals^T + bo1) + bo2
        mip = ctx.enter_context(tc.tile_pool(name="mip", bufs=2))
        outp = ctx.enter_context(tc.tile_pool(name="outp", bufs=2))
        psum2s = [ps.tile([128, NQ], f32, tag="ps", name=f"p2_{m}") for m in range(MO)]
        for m in range(MH):
            p1 = ps.tile([128, NQ], f32, tag="ps")
            for c in range(MH):
                nc.tensor.matmul(
                    p1[:], wo1_s[:, c * HD + m * 128:c * HD + (m + 1) * 128],
                    valsT_s[:, c * NQ:(c + 1) * NQ], start=(c == 0), stop=(c == MH - 1))
            # mish(x) = x*(u^2+2u)/(u^2+2u+2), u = e^x  (no Mish LUT on this arch)
            xm = mip.tile([128, NQ], f32, tag="xm")
            nc.scalar.activation(xm[:], p1[:], Ident, bias=bo1_s[:, m:m + 1])
            um = mip.tile([128, NQ], f32, tag="um")
            nc.scalar.activation(um[:], p1[:], Exp, bias=bo1_s[:, m:m + 1])
            wm = mip.tile([128, NQ], f32, tag="wm")
            nc.vector.scalar_tensor_tensor(
                wm[:], um[:], 2.0, um[:], op0=mybir.AluOpType.add, op1=mybir.AluOpType.mult)
            dm = mip.tile([128, NQ], f32, tag="dm")
            nc.vector.tensor_scalar_add(dm[:], wm[:], 2.0)
            rm = mip.tile([128, NQ], f32, tag="rm")
            nc.vector.reciprocal(rm[:], dm[:])
            tm = mip.tile([128, NQ], f32, tag="tm")
            nc.vector.tensor_mul(tm[:], wm[:], rm[:])
            mi = mip.tile([128, NQ], bf16, tag="mi")
            nc.vector.tensor_mul(mi[:], xm[:], tm[:])
            for mo in range(MO):
                nc.tensor.matmul(
                    psum2s[mo][:], wo2_s[:, m * DIN + mo * 128:m * DIN + (mo + 1) * 128], mi[:],
                    start=(m == 0), stop=(m == MH - 1))
        for mo in range(MO):
            ot = outp.tile([128, NQ], f32, tag="ot")
            nc.scalar.activation(ot[:], psum2s[mo][:], Ident, bias=bo2_s[:, mo:mo + 1])
            nc.sync.dma_start(out[mo * 128:(mo + 1) * 128, :], ot[:])

        if dbg is not None:
            nc.sync.dma_start(dbg["qt_d"][:], QT_s[:])
            nc.sync.dma_start(dbg["v_d"][:], V_s[:])
            nc.sync.dma_start(dbg["eb_d"][:], eb_s[:])
            nc.sync.dma_start(dbg["vals_d"][:], vals_s[:])
            nc.sync.dma_start(dbg["valst_d"][:], valsT_s[:])
            nc.sync.dma_start(dbg["ktd_d"][:], ktd[:])


def build(debug_outs=False, amplify=1):
    nc = bacc.Bacc("TRN2", target_bir_lowering=False, debug=False, num_devices=NCORES)
    A = {}

    def din(name, shape, dt=bf16):
        A[name] = nc.dram_tensor(name, shape, dt, kind="ExternalInput").ap()

    din("qT", [DIN, NQ])
    din("kT", [DIN, N])
    din("vT", [DIN, N])
    din("bT", [N, NQ])
    din("mT", [N, NQ])
    din("wq", [DIN, HD])
    din("wk", [DIN, HD])
    din("wv", [DIN, HD])
    din("wo1", [HD, HD])
    din("wo2", [HD, DIN])
    din("bo1", [HD, 1], f32)
    din("bo2", [DIN, 1], f32)
    out = nc.dram_tensor("out", [DIN, NQ], f32, kind="ExternalOutput").ap()
    dbg = None
    if debug_outs:
        dbg = {
            "qt_d": nc.dram_tensor("qt_d", [128, MH * NQ], bf16, kind="ExternalOutput").ap(),
            "v_d": nc.dram_tensor("v_d", [128, KC * H * 65], bf16, kind="ExternalOutput").ap(),
            "eb_d": nc.dram_tensor("eb_d", [128, KC * NQ], bf16, kind="ExternalOutput").ap(),
            "vals_d": nc.dram_tensor("vals_d", [128, QTILES * HD], bf16, kind="ExternalOutput").ap(),
            "valst_d": nc.dram_tensor("valst_d", [128, MH * NQ], bf16, kind="ExternalOutput").ap(),
            "ktd_d": nc.dram_tensor("ktd_d", [HD, N], bf16, kind="ExternalOutput").ap(),
            "lg_d": nc.dram_tensor("lg_d", [128, NQ], f32, kind="ExternalOutput").ap(),
            "am_d": nc.dram_tensor("am_d", [128, NQ], bf16, kind="ExternalOutput").ap(),
            "av_d": nc.dram_tensor("av_d", [128, 65], f32, kind="ExternalOutput").ap(),
        }
    with tile.TileContext(nc) as tc:
        _body(tc, A, out, dbg, amplify=amplify)
    nc.compile()
    return nc


_CACHE = {}


def _get_nc():
    if "nc" not in _CACHE:
        _CACHE["nc"] = build()
    return _CACHE["nc"]


def make_in_maps(q, k, v, bias, mask, Wq, bq, Wk, bk, Wv, bv, Wo1, bo1, Wo2, bo2):
    q = np.asarray(q, np.float32)
    k = np.asarray(k, np.float32)
    v = np.asarray(v, np.float32)
    bias = np.asarray(bias, np.float32)
    scale = 1.0 / math.sqrt(Dh)
    wq_h = np.ascontiguousarray((np.asarray(Wq, np.float32) * scale).T).astype(BF)
    wk_h = np.ascontiguousarray(np.asarray(Wk, np.float32).T).astype(BF)
    wv_h = np.ascontiguousarray(np.asarray(Wv, np.float32).T).astype(BF)
    wo1_h = np.ascontiguousarray(np.asarray(Wo1, np.float32).T).astype(BF)
    wo2_h = np.ascontiguousarray(np.asarray(Wo2, np.float32).T).astype(BF)
    bo1_h = np.ascontiguousarray(np.asarray(bo1, np.float32).reshape(HD, 1))
    bo2_h = np.ascontiguousarray(np.asarray(bo2, np.float32).reshape(DIN, 1))
    kT_h = np.ascontiguousarray(k.T).astype(BF)
    vT_h = np.ascontiguousarray(v.T).astype(BF)
    maskf = np.asarray(mask).astype(np.float32)
    in_maps = []
    for c in range(NCORES):
        sl = slice(c * NQ, (c + 1) * NQ)
        in_maps.append({
            "qT": np.ascontiguousarray(q[sl].T).astype(BF),
            "kT": kT_h,
            "vT": vT_h,
            "bT": np.ascontiguousarray(bias[sl].T).astype(BF),
            "mT": np.ascontiguousarray(maskf[sl].T).astype(BF),
            "wq": wq_h, "wk": wk_h, "wv": wv_h, "wo1": wo1_h, "wo2": wo2_h,
            "bo1": bo1_h, "bo2": bo2_h,
        })
    return in_maps


def gather(results, bias):
    out = np.empty((N, DIN), np.float32)
    for c in range(NCORES):
        out[c * NQ:(c + 1) * NQ, :] = results[c]["out"].T
    return out, np.asarray(bias)


def kernel(q, k, v, bias, mask, Wq, bq, Wk, bk, Wv, bv, Wo1, bo1, Wo2, bo2):
    nc = _get_nc()
    in_maps = make_in_maps(q, k, v, bias, mask, Wq, bq, Wk, bk, Wv, bv, Wo1, bo1, Wo2, bo2)
    res = run_bass_kernel_spmd(nc, in_maps, core_ids=list(range(NCORES)))
    return gather(res.results, bias)


# revision 15
# speedup vs baseline: 1.1331x; 1.1331x over previous
"""Trainium2 Bass kernel for CrAKNAttention (N=3072, DIN=512, H=16, D=64) on 8 NeuronCores.

Sharding: queries are split 8 ways (384 rows per core); every core computes all 16
heads for its query slice plus the full K/V projections, so no collectives are
needed. All device matmuls run in bf16 with f32 PSUM accumulation.

Layout notes (everything transposed so the PE contraction dim is the partition dim):
 - host passes qT/kT/vT [DIN, nq|N], bias^T/mask^T slices [N, nq], and transposed
   weights; 1/sqrt(D) is folded into Wq on the host.
 - QK^T is computed key-major ([128 keys, 384 queries] tiles); softmax numerator is
   exp(QK) * (exp(bias) * mask) -- exact since logits are bounded far below exp
   overflow and masked entries multiply to 0.
 - attn^T tiles feed the AV matmul directly as lhsT; a ones-column matmul into the
   same PSUM bank accumulates the softmax denominator.
 - o_proj runs on vals^T (Mish is a native ScalarE activation); the core writes the
   transposed output slice [512, 384], which the host transposes back and concats.
"""

import math
import sys

sys.path.insert(0, "/opt/trn_rl_repo")

import numpy as np
import ml_dtypes

import concourse.bass as bass  # noqa: F401  (registers engines)
import concourse.mybir as mybir
import concourse.tile as tile
from concourse import bacc
from concourse.bass_utils import run_bass_kernel_spmd
from concourse.masks import make_identity

f32 = mybir.dt.float32
bf16 = mybir.dt.bfloat16
BF = ml_dtypes.bfloat16

N, DIN, H, Dh = 3072, 512, 16, 64
HD = H * Dh  # 1024
NCORES = 8
NQ = N // NCORES  # 384 queries per core
QTILES = NQ // 128  # 3
KC = N // 128  # 24 key chunks
DC = DIN // 128  # 4 din chunks
MH = HD // 128  # 8 hd tiles
MO = DIN // 128  # 4 output tiles


def _body(tc, A, out, dbg=None, amplify=1):
    nc = tc.nc
    Exp = mybir.ActivationFunctionType.Exp
    Mish = mybir.ActivationFunctionType.Mish
    Ident = mybir.ActivationFunctionType.Identity
    from contextlib import ExitStack

    with ExitStack() as ctx:
        res = ctx.enter_context(tc.tile_pool(name="res", bufs=1))
        dram = ctx.enter_context(tc.tile_pool(name="dram", bufs=1, space="DRAM"))
        ps = ctx.enter_context(tc.tile_pool(name="ps", bufs=8, space="PSUM"))

        # resident SBUF tiles
        wq_s = res.tile([128, DC * HD], bf16, tag="wq")
        wk_s = res.tile([128, DC * HD], bf16, tag="wk")
        wv_s = res.tile([128, DC * HD], bf16, tag="wv")
        wo1_s = res.tile([128, MH * HD], bf16, tag="wo1")
        wo2_s = res.tile([128, MH * DIN], bf16, tag="wo2")
        QT_s = res.tile([128, MH * NQ], bf16, tag="qt")
        V_s = res.tile([128, KC * H * 65], bf16, tag="v")
        eb_s = res.tile([128, KC * NQ], bf16, tag="eb")
        vals_s = res.tile([128, QTILES * HD], bf16, tag="vals")
        valsT_s = res.tile([128, MH * NQ], bf16, tag="valsT")
        bo1_s = res.tile([128, MH], f32, tag="bo1")
        bo2_s = res.tile([128, MO], f32, tag="bo2")
        ident = res.tile([128, 128], bf16, tag="ident")
        ones_s = res.tile([128, 1], bf16, tag="ones")
        ktd = dram.tile([HD, N], bf16)

        make_identity(nc, ident[:])
        nc.vector.memset(ones_s[:], 1.0)
        nc.vector.memset(V_s[:], 1.0)  # ones columns for the softmax denominator

        for c in range(DC):
            nc.sync.dma_start(wq_s[:, c * HD:(c + 1) * HD], A["wq"][c * 128:(c + 1) * 128, :])
            nc.sync.dma_start(wk_s[:, c * HD:(c + 1) * HD], A["wk"][c * 128:(c + 1) * 128, :])
            nc.sync.dma_start(wv_s[:, c * HD:(c + 1) * HD], A["wv"][c * 128:(c + 1) * 128, :])
        for c in range(MH):
            nc.sync.dma_start(wo1_s[:, c * HD:(c + 1) * HD], A["wo1"][c * 128:(c + 1) * 128, :])
            nc.sync.dma_start(wo2_s[:, c * DIN:(c + 1) * DIN], A["wo2"][c * 128:(c + 1) * 128, :])
        nc.sync.dma_start(bo1_s[:], A["bo1"].rearrange("(m p) o -> p (m o)", p=128))
        nc.sync.dma_start(bo2_s[:], A["bo2"].rearrange("(m p) o -> p (m o)", p=128))

        # exp(bias)*mask, key-major [128k, NQ] per key chunk
        ebp = ctx.enter_context(tc.tile_pool(name="ebp", bufs=2))
        for kc in range(KC):
            bt = ebp.tile([128, NQ], bf16, tag="bt")
            mt = ebp.tile([128, NQ], bf16, tag="mt")
            nc.sync.dma_start(bt[:], A["bT"][kc * 128:(kc + 1) * 128, :])
            nc.sync.dma_start(mt[:], A["mT"][kc * 128:(kc + 1) * 128, :])
            et = ebp.tile([128, NQ], bf16, tag="et")
            nc.scalar.activation(et[:], bt[:], Exp)
            nc.vector.tensor_mul(eb_s[:, kc * NQ:(kc + 1) * NQ], et[:], mt[:])

        # q projection -> QT_s [hd, q] (hd-major)
        qp = ctx.enter_context(tc.tile_pool(name="qp", bufs=DC))
        qrs = []
        for c in range(DC):
            qr = qp.tile([128, NQ], bf16, tag="qr", name=f"qr{c}")
            nc.sync.dma_start(qr[:], A["qT"][c * 128:(c + 1) * 128, :])
            qrs.append(qr)
        for m in range(MH):
            qps = ps.tile([128, 512], f32, tag="ps", name=f"qps{m}")
            for c in range(DC):
                nc.tensor.matmul(
                    qps[:, 0:NQ], wq_s[:, c * HD + m * 128:c * HD + (m + 1) * 128],
                    qrs[c][:], start=(c == 0), stop=(c == DC - 1))
            nc.scalar.copy(QT_s[:, m * NQ:(m + 1) * NQ], qps[:, 0:NQ])

        # k projection -> ktd DRAM [hd, tok] (hd-major)
        kp = ctx.enter_context(tc.tile_pool(name="kp", bufs=2 * DC))
        kst = ctx.enter_context(tc.tile_pool(name="kst", bufs=3))
        vp = ctx.enter_context(tc.tile_pool(name="vp", bufs=2 * DC))
        for n in range(N // 512):
            krs, vts = [], []
            for c in range(DC):
                kr = kp.tile([128, 512], bf16, tag="kr", name=f"kr{n}_{c}")
                nc.sync.dma_start(kr[:], A["kT"][c * 128:(c + 1) * 128, n * 512:(n + 1) * 512])
                krs.append(kr)
                vt = vp.tile([128, 512], bf16, tag="vt", name=f"vt{n}_{c}")
                nc.sync.dma_start(vt[:], A["vT"][c * 128:(c + 1) * 128, n * 512:(n + 1) * 512])
                vts.append(vt)
            for m in range(MH):
                kps = ps.tile([128, 512], f32, tag="ps", name=f"kps{n}_{m}")
                for c in range(DC):
                    nc.tensor.matmul(
                        kps[:], wk_s[:, c * HD + m * 128:c * HD + (m + 1) * 128], krs[c][:],
                        start=(c == 0), stop=(c == DC - 1))
                ks = kst.tile([128, 512], bf16, tag="ks")
                nc.scalar.copy(ks[:], kps[:])
                nc.sync.dma_start(ktd[m * 128:(m + 1) * 128, n * 512:(n + 1) * 512], ks[:])
            for j in range(4):
                mt = n * 4 + j
                for half in range(2):
                    pv = ps.tile([128, 512], f32, tag="ps", name=f"pv{n}_{j}_{half}")
                    for c in range(DC):
                        nc.tensor.matmul(
                            pv[:], vts[c][:, j * 128:(j + 1) * 128],
                            wv_s[:, c * HD + half * 512:c * HD + half * 512 + 512],
                            start=(c == 0), stop=(c == DC - 1))
                    dst = V_s[:].rearrange("p (a h e) -> p a h e", a=KC, h=H)[
                        :, mt, half * 8:(half + 1) * 8, 0:64]
                    nc.vector.tensor_copy(dst, pv[:].rearrange("p (h e) -> p h e", h=8))

        # attention, head by head
        kwp = ctx.enter_context(tc.tile_pool(name="kwp", bufs=2))
        atp = ctx.enter_context(tc.tile_pool(name="atp", bufs=3))
        rcp = ctx.enter_context(tc.tile_pool(name="rcp", bufs=2))
        for rep in range(amplify):
          for h in range(H):
            avs = [ps.tile([128, 65], f32, tag="ps", name=f"av{rep}_{h}_{q}") for q in range(QTILES)]
            qrhs = QT_s[(h % 2) * 64:(h % 2) * 64 + 64, (h // 2) * NQ:(h // 2 + 1) * NQ]
            off = (h % 2) * 64
            kht = kwp.tile([128, N], bf16, tag="kht", name=f"kht{rep}_{h}")
            nc.sync.dma_start(kht[off:off + 64, :], ktd[h * 64:h * 64 + 64, :])
            for kc in range(KC):
                pqk = ps.tile([128, NQ], f32, tag="ps")
                nc.tensor.matmul(
                    pqk[:], kht[off:off + 64, kc * 128:(kc + 1) * 128], qrhs,
                    start=True, stop=True)
                at = atp.tile([128, NQ], bf16, tag="at")
                nc.scalar.activation(at[:], pqk[:], Exp)
                am = atp.tile([128, NQ], bf16, tag="am")
                nc.vector.tensor_mul(am[:], at[:], eb_s[:, kc * NQ:(kc + 1) * NQ])
                if dbg is not None and h == 0 and kc == 0:
                    lgs = atp.tile([128, NQ], f32, tag="lgs")
                    nc.vector.tensor_copy(lgs[:], pqk[:])
                    nc.sync.dma_start(dbg["lg_d"][:], lgs[:])
                    nc.sync.dma_start(dbg["am_d"][:], am[:])
                first, last = kc == 0, kc == KC - 1
                vslice = V_s[:, kc * H * 65 + h * 65:kc * H * 65 + h * 65 + 65]
                for qt in range(QTILES):
                    nc.tensor.matmul(
                        avs[qt][:, 0:65], am[:, qt * 128:(qt + 1) * 128], vslice,
                        start=first, stop=last)
            if dbg is not None and h == 0:
                avd = atp.tile([128, 65], f32, tag="avd")
                nc.vector.tensor_copy(avd[:], avs[0][:])
                nc.sync.dma_start(dbg["av_d"][:], avd[:])
            for qt in range(QTILES):
                rc = rcp.tile([128, 1], f32, tag="rc")
                nc.vector.reciprocal(rc[:], avs[qt][:, 64:65])
                nc.vector.tensor_scalar_mul(
                    vals_s[:, qt * HD + h * 64:qt * HD + h * 64 + 64], avs[qt][:, 0:64], rc[:])

        # vals [q, hd] -> valsT [hd, q] via PE transpose
        for qt in range(QTILES):
            for m in range(MH):
                pt = ps.tile([128, 128], bf16, tag="ps")
                nc.tensor.transpose(pt[:], vals_s[:, qt * HD + m * 128:qt * HD + (m + 1) * 128], ident[:])
                nc.scalar.copy(valsT_s[:, m * NQ + qt * 128:m * NQ + (qt + 1) * 128], pt[:])

        # o_proj: out^T = Wo2 @ mish(Wo1 @ vals^T + bo1) + bo2
        mip = ctx.enter_context(tc.tile_pool(name="mip", bufs=2))
        outp = ctx.enter_context(tc.tile_pool(name="outp", bufs=2))
        psum2s = [ps.tile([128, NQ], f32, tag="ps", name=f"p2_{m}") for m in range(MO)]
        for m in range(MH):
            p1 = ps.tile([128, NQ], f32, tag="ps")
            for c in range(MH):
                nc.tensor.matmul(
                    p1[:], wo1_s[:, c * HD + m * 128:c * HD + (m + 1) * 128],
                    valsT_s[:, c * NQ:(c + 1) * NQ], start=(c == 0), stop=(c == MH - 1))
            # mish(x) = x*(u^2+2u)/(u^2+2u+2), u = e^x  (no Mish LUT on this arch)
            xm = mip.tile([128, NQ], f32, tag="xm")
            nc.scalar.activation(xm[:], p1[:], Ident, bias=bo1_s[:, m:m + 1])
            um = mip.tile([128, NQ], f32, tag="um")
            nc.scalar.activation(um[:], p1[:], Exp, bias=bo1_s[:, m:m + 1])
            wm = mip.tile([128, NQ], f32, tag="wm")
            nc.vector.scalar_tensor_tensor(
                wm[:], um[:], 2.0, um[:], op0=mybir.AluOpType.add, op1=mybir.AluOpType.mult)
            dm = mip.tile([128, NQ], f32, tag="dm")
            nc.vector.tensor_scalar_add(dm[:], wm[:], 2.0)
            rm = mip.tile([128, NQ], f32, tag="rm")
            nc.vector.reciprocal(rm[:], dm[:])
            tm = mip.tile([128, NQ], f32, tag="tm")
            nc.vector.tensor_mul(tm[:], wm[:], rm[:])
            mi = mip.tile([128, NQ], bf16, tag="mi")
            nc.vector.tensor_mul(mi[:], xm[:], tm[:])
            for mo in range(MO):
                nc.tensor.matmul(
                    psum2s[mo][:], wo2_s[:, m * DIN + mo * 128:m * DIN + (mo + 1) * 128], mi[:],
                    start=(m == 0), stop=(m == MH - 1))
        for mo in range(MO):
            ot = outp.tile([128, NQ], f32, tag="ot")
            nc.scalar.activation(ot[:], psum2s[mo][:], Ident, bias=bo2_s[:, mo:mo + 1])
            nc.sync.dma_start(out[mo * 128:(mo + 1) * 128, :], ot[:])

        if dbg is not None:
            nc.sync.dma_start(dbg["qt_d"][:], QT_s[:])
            nc.sync.dma_start(dbg["v_d"][:], V_s[:])
            nc.sync.dma_start(dbg["eb_d"][:], eb_s[:])
            nc.sync.dma_start(dbg["vals_d"][:], vals_s[:])
            nc.sync.dma_start(dbg["valst_d"][:], valsT_s[:])
            nc.sync.dma_start(dbg["ktd_d"][:], ktd[:])


def build(debug_outs=False, amplify=1):
    nc = bacc.Bacc("TRN2", target_bir_lowering=False, debug=False, num_devices=NCORES)
    A = {}

    def din(name, shape, dt=bf16):
        A[name] = nc.dram_tensor(name, shape, dt, kind="ExternalInput").ap()

    din("qT", [DIN, NQ])
    din("kT", [DIN, N])
    din("vT", [DIN, N])
    din("bT", [N, NQ])
    din("mT", [N, NQ])
    din("wq", [DIN, HD])
    din("wk", [DIN, HD])
    din("wv", [DIN, HD])
    din("wo1", [HD, HD])
    din("wo2", [HD, DIN])
    din("bo1", [HD, 1], f32)
    din("bo2", [DIN, 1], f32)
    out = nc.dram_tensor("out", [DIN, NQ], f32, kind="ExternalOutput").ap()
    dbg = None
    if debug_outs:
        dbg = {
            "qt_d": nc.dram_tensor("qt_d", [128, MH * NQ], bf16, kind="ExternalOutput").ap(),
            "v_d": nc.dram_tensor("v_d", [128, KC * H * 65], bf16, kind="ExternalOutput").ap(),
            "eb_d": nc.dram_tensor("eb_d", [128, KC * NQ], bf16, kind="ExternalOutput").ap(),
            "vals_d": nc.dram_tensor("vals_d", [128, QTILES * HD], bf16, kind="ExternalOutput").ap(),
            "valst_d": nc.dram_tensor("valst_d", [128, MH * NQ], bf16, kind="ExternalOutput").ap(),
            "ktd_d": nc.dram_tensor("ktd_d", [HD, N], bf16, kind="ExternalOutput").ap(),
            "lg_d": nc.dram_tensor("lg_d", [128, NQ], f32, kind="ExternalOutput").ap(),
            "am_d": nc.dram_tensor("am_d", [128, NQ], bf16, kind="ExternalOutput").ap(),
            "av_d": nc.dram_tensor("av_d", [128, 65], f32, kind="ExternalOutput").ap(),
        }
    with tile.TileContext(nc) as tc:
        _body(tc, A, out, dbg, amplify=amplify)
    nc.compile()
    return nc


_CACHE = {}


def _get_nc():
    if "nc" not in _CACHE:
        _CACHE["nc"] = build()
    return _CACHE["nc"]


def make_in_maps(q, k, v, bias, mask, Wq, bq, Wk, bk, Wv, bv, Wo1, bo1, Wo2, bo2):
    q = np.asarray(q, np.float32)
    k = np.asarray(k, np.float32)
    v = np.asarray(v, np.float32)
    bias = np.asarray(bias, np.float32)
    scale = 1.0 / math.sqrt(Dh)
    wq_h = np.ascontiguousarray((np.asarray(Wq, np.float32) * scale).T).astype(BF)
    wk_h = np.ascontiguousarray(np.asarray(Wk, np.float32).T).astype(BF)
    wv_h = np.ascontiguousarray(np.asarray(Wv, np.float32).T).astype(BF)
    wo1_h = np.ascontiguousarray(np.asarray(Wo1, np.float32).T).astype(BF)
    wo2_h = np.ascontiguousarray(np.asarray(Wo2, np.float32).T).astype(BF)
    bo1_h = np.ascontiguousarray(np.asarray(bo1, np.float32).reshape(HD, 1))
    bo2_h = np.ascontiguousarray(np.asarray(bo2, np.float32).reshape(DIN, 1))
    kT_h = np.ascontiguousarray(k.T).astype(BF)
    vT_h = np.ascontiguousarray(v.T).astype(BF)
    maskf = np.asarray(mask).astype(np.float32)
    in_maps = []
    for c in range(NCORES):
        sl = slice(c * NQ, (c + 1) * NQ)
        in_maps.append({
            "qT": np.ascontiguousarray(q[sl].T).astype(BF),
            "kT": kT_h,
            "vT": vT_h,
            "bT": np.ascontiguousarray(bias[sl].T).astype(BF),
            "mT": np.ascontiguousarray(maskf[sl].T).astype(BF),
            "wq": wq_h, "wk": wk_h, "wv": wv_h, "wo1": wo1_h, "wo2": wo2_h,
            "bo1": bo1_h, "bo2": bo2_h,
        })
    return in_maps


def gather(results, bias):
    out = np.empty((N, DIN), np.float32)
    for c in range(NCORES):
        out[c * NQ:(c + 1) * NQ, :] = results[c]["out"].T
    return out, np.asarray(bias)


def kernel(q, k, v, bias, mask, Wq, bq, Wk, bk, Wv, bv, Wo1, bo1, Wo2, bo2):
    nc = _get_nc()
    in_maps = make_in_maps(q, k, v, bias, mask, Wq, bq, Wk, bk, Wv, bv, Wo1, bo1, Wo2, bo2)
    res = run_bass_kernel_spmd(nc, in_maps, core_ids=list(range(NCORES)))
    return gather(res.results, bias)
